# revision 1
# baseline (speedup 1.0000x reference)
"""Trainium2 Bass kernel for nn_Cheb_35888746725726 (ChebConv K=3 GNN, N=50000,
E=800000, F=H=96, lambda_max=2 -> diag term is 0).

Strategy (8 NeuronCores, node/graph-parallel):
 - Host: compute Chebyshev edge norm (deg/rsqrt/norm_w), degree-balanced node
   permutation into 784 tiles of 64 nodes (padded N=50176 = 8 cores x 6272),
   shard edges by destination core, sort per 64-node dst tile, pad each tile's
   edge list to 8x128 slots. Ship per core only: the core's feature-major x
   shard, per-edge (src id, dst slot, weight), and the small dense weights —
   packed into TWO device arrays (bf16 blob + int32 blob with f32 bitcast
   columns) to minimize host->device transfer time.
 - Device: build the per-edge-tile *weighted one-hot* scatter matrices
   (128 edges x 64 dst-slots, bf16) in SBUF from (slot, weight) via fused
   is_equal/mult tensor_scalar against an iota row; AllGather the x shards
   into the full gather table. Per prop: indirect-DMA gather of source rows
   (bf16) from the HBM node table -> scatter via one-hot matmuls accumulating
   in PSUM. Chebyshev recurrence folded into host-modified dense weights:
   out = Tx0 @ (W0-W2) + Tx1 @ W1 + (L@Tx1) @ (2*W2), so Tx2 is never formed.
 - AllGather (8 cores) rebuilds the full node table between dependent props.
 - Dense 96x96 matmuls run feature-major; PE transposes convert layouts.
"""
import numpy as np
import ml_dtypes

import jax

# Persistent XLA compilation cache: run_bass_kernel_spmd re-jits on every
# call, so without this each warm call re-runs the BIR verify/NEFF compile
# (~1s). With it, identical HLO hits the disk cache.
try:
    jax.config.update("jax_compilation_cache_dir", "/tmp/jax_comp_cache")
    jax.config.update("jax_persistent_cache_min_compile_time_secs", 0.0)
    jax.config.update("jax_persistent_cache_min_entry_size_bytes", 0)
except Exception:
    pass

import concourse.bass as bass
import concourse.bacc as bacc
import concourse.mybir as mybir
import concourse.tile as tile
from concourse.bass import ds
from concourse.bass_utils import run_bass_kernel_spmd
from concourse.masks import make_identity

# ---- problem constants (hardcoded per the harness contract) ----
N = 50000
E = 800000
F = 96
K = 3
C = 8                    # cores
NP_PAD = 50176           # 8 * 6272
SHARD = NP_PAD // C      # 6272
NTW = 64                 # node-tile width
NT = SHARD // NTW        # 98 node tiles / core
TE = 8                   # edge tiles (of 128 edges) per node tile
P = 128
NCHUNK = 512             # dense matmul node-chunk

# blob column offsets (bf16 [128, CB]); narrow-dtype regions (uint16 gather
# indices, uint8 slots, f32 biases) are bitcast into bf16 columns so
# everything ships as ONE device array.
# Rows 96:128 of the x region hide the per-edge (weight bf16, slot u8) table:
# lane p = q*32+r lives at blob[96+r, q*1568 : (q+1)*1568] with w in
# stripe cols [0:784] and slot-u8 in stripe cols [784:1176] (bitcast).
NTT = NT * TE            # 784 edge tiles per core
STRIPE = SHARD // 4      # 1568 fold-stripe width
OFF_XT = 0               # [0:96, 0:SHARD] feature-major x shard
OFF_SRC = SHARD          # [128, NTT] uint16 gather indices (bitcast)
OFF_W = OFF_SRC + NTT    # [0:96, 578] six folded WK mats + wlin
OFF_B = OFF_W + 6 * F + 2         # [0:96, 6] b1,b2,blin as f32 bitcast
CB = OFF_B + 6
assert OFF_B % 2 == 0 and CB % 2 == 0    # 4-byte alignment for the f32 bitcast
assert NTT + NTT // 2 <= STRIPE          # w + slot-u8 fit in each fold stripe

BF = ml_dtypes.bfloat16

import os
DBG_NO_AG = bool(int(os.environ.get("KDBG_NO_AG", "0")))     # replace collectives with local copies
DBG_CORES = int(os.environ.get("KDBG_CORES", str(C)))         # cores to run

_compiled = None         # cache (nc, meta) across calls


# --------------------------------------------------------------------------
# host-side preprocessing
# --------------------------------------------------------------------------
def _preprocess(x, edge_index, edge_weight):
    src = np.asarray(edge_index[0]).astype(np.int64)
    dst = np.asarray(edge_index[1]).astype(np.int64)
    w = np.asarray(edge_weight).astype(np.float32)

    deg = np.zeros(N, np.float32)
    np.add.at(deg, src, w)
    dis = np.where(deg > 0, 1.0 / np.sqrt(np.maximum(deg, 1e-30)), 0.0).astype(np.float32)
    norm_w = (-dis[src] * w * dis[dst]).astype(np.float32)

    # degree-balanced assignment of nodes to 784 tiles of 64 (LPT greedy)
    indeg = np.bincount(dst, minlength=N).astype(np.int64)
    n_tiles = NP_PAD // NTW
    order = np.argsort(-indeg, kind="stable")
    import heapq
    heap = [(0, 0, t) for t in range(n_tiles)]
    heapq.heapify(heap)
    tile_assign = np.empty(N, np.int64)
    for n in order:
        while True:
            load, cnt, t = heapq.heappop(heap)
            if cnt < NTW:
                tile_assign[n] = t
                heapq.heappush(heap, (load + indeg[n], cnt + 1, t))
                break
            # full tile: drop from heap permanently
    # slot within tile: rank of node among nodes of the same tile (by node id)
    order2 = np.argsort(tile_assign, kind="stable")        # groups nodes by tile
    slot_in_tile = np.empty(N, np.int64)
    counts = np.bincount(tile_assign, minlength=n_tiles)
    starts = np.concatenate([[0], np.cumsum(counts)[:-1]])
    slot_in_tile[order2] = np.arange(N) - np.repeat(starts, counts)
    new_id = tile_assign * NTW + slot_in_tile

    src_n = new_id[src]
    dst_n = new_id[dst]

    # bucket every edge into (core, 128-edge tile column, lane) in one pass:
    # edges sorted by destination tile; rank within tile decides lane/column
    o = np.argsort(dst_n, kind="stable")
    es, ed, ew = src_n[o], dst_n[o], norm_w[o]
    gtile = ed // NTW                                      # global 64-node tile id
    tstart = np.searchsorted(gtile, np.arange(n_tiles))
    r = np.arange(E) - tstart[gtile]                       # rank within tile
    assert r.max() < TE * P, f"tile overflow: {r.max() + 1}"
    core = gtile // NT
    col = (gtile % NT) * TE + r // P
    lane = r % P

    src_idx = np.zeros((C, P, NT * TE), np.int32)
    slot_a = np.zeros((C, P, NT * TE), np.float32)
    w_a = np.zeros((C, P, NT * TE), np.float32)
    src_idx[core, lane, col] = es
    slot_a[core, lane, col] = ed - gtile * NTW
    w_a[core, lane, col] = ew

    return new_id, src_idx, slot_a, w_a


# --------------------------------------------------------------------------
# bass kernel builder
# --------------------------------------------------------------------------
def _build_kernel():
    dt = mybir.dt
    nc = bacc.Bacc("TRN2", target_bir_lowering=False, debug=False, num_devices=DBG_CORES)

    blob_d = nc.dram_tensor("blob", [P, CB], dt.bfloat16, kind="ExternalInput")
    out_d = nc.dram_tensor("out", [2, SHARD], dt.bfloat16, kind="ExternalOutput")

    rg = [list(range(C))]

    with tile.TileContext(nc) as tc:
        with (
            tc.tile_pool(name="res", bufs=1) as res,          # resident sbuf
            tc.tile_pool(name="mpool", bufs=4) as mpool,      # gather dests
            tc.tile_pool(name="spool", bufs=2) as spool,      # small evac tiles
            tc.tile_pool(name="pscat", bufs=4, space="PSUM") as pscat,
            tc.tile_pool(name="ptr", bufs=2, space="PSUM") as ptr,
            tc.tile_pool(name="pdense", bufs=2, space="PSUM") as pdense,
            tc.tile_pool(name="dram", bufs=1, space="DRAM") as dram,
        ):
            # ---------- resident loads ----------
            # feature-major activation buffers (bf16)
            fm = {
                "tx0": res.tile([F, SHARD], dt.bfloat16, name="fm_tx0"),
                "t1": res.tile([F, SHARD], dt.bfloat16, name="fm_t1"),
                "s2": res.tile([F, SHARD], dt.bfloat16, name="fm_s2"),
                "h": res.tile([F, SHARD], dt.bfloat16, name="fm_h"),
            }
            nc.sync.dma_start(out=fm["tx0"][:], in_=blob_d[0:F, OFF_XT:OFF_XT + SHARD])

            ew_sb = res.tile([P, NTT], dt.bfloat16)      # edge weights
            slot8_sb = res.tile([P, NTT], dt.uint8)      # dst slots (u8)
            for q in range(4):
                st = q * STRIPE
                nc.sync.dma_start(
                    out=ew_sb[q * 32:(q + 1) * 32, :],
                    in_=blob_d[F:P, st:st + NTT])
                nc.sync.dma_start(
                    out=slot8_sb[q * 32:(q + 1) * 32, :],
                    in_=blob_d[F:P, st + NTT:st + NTT + NTT // 2].bitcast(dt.uint8))
            slot_sb = res.tile([P, NTT], dt.bfloat16)
            nc.vector.tensor_copy(out=slot_sb[:], in_=slot8_sb[:])
            w_sb = res.tile([F, 6 * F + 2], dt.bfloat16)
            nc.sync.dma_start(out=w_sb[:], in_=blob_d[0:F, OFF_W:OFF_W + 6 * F + 2])
            src16_sb = res.tile([P, NTT], dt.uint16)
            nc.sync.dma_start(
                out=src16_sb[:],
                in_=blob_d[:, OFF_SRC:OFF_SRC + NTT].bitcast(dt.uint16))
            src_sb = res.tile([P, NTT], dt.int32)
            nc.vector.tensor_copy(out=src_sb[:], in_=src16_sb[:])
            bias_sb = res.tile([F, 2], dt.float32)
            nc.sync.dma_start(
                out=bias_sb[:],
                in_=blob_d[0:F, OFF_B:OFF_B + 4].bitcast(dt.float32))
            blin_sb = res.tile([2, 1], dt.float32)
            nc.sync.dma_start(
                out=blin_sb[:],
                in_=blob_d[0:2, OFF_B + 4:OFF_B + 6].bitcast(dt.float32))
            ident = res.tile([P, P], dt.bfloat16)
            make_identity(nc, ident[:])

            # iota row 0..63 repeated on every partition (for one-hot build)
            iota_i = res.tile([P, NTW], dt.int32)
            nc.gpsimd.iota(iota_i[:], pattern=[[1, NTW]], base=0, channel_multiplier=0)
            iota_b = res.tile([P, NTW], dt.bfloat16)
            nc.vector.tensor_copy(out=iota_b[:], in_=iota_i[:])

            # node-major staging for table writes / transposes
            s_nm = res.tile([P, (NT // 2) * F], dt.bfloat16)

            # internal DRAM
            bounce = [dram.tile([SHARD, F], dt.bfloat16, name=f"bounce{i}") for i in range(4)]
            ag = [dram.tile([NP_PAD, F], dt.bfloat16,
                            addr_space=("Local" if DBG_NO_AG else "Shared"), name=f"ag{i}")
                  for i in range(4)]

            # one-hot scatter matrices, built on device with broadcast views:
            # oh[p, t*64 + s] = w[p, t] * (slot[p, t] == s)
            oh_sb = res.tile([P, NT * TE * NTW], dt.bfloat16)
            oh_v = oh_sb[:].rearrange("p (t f) -> p t f", f=NTW)
            iota_v = iota_b[:].rearrange("p (one f) -> p one f", one=1) \
                              .broadcast_to((P, NTT, NTW))
            slot_v = slot_sb[:].rearrange("p (t one) -> p t one", one=1) \
                               .broadcast_to((P, NTT, NTW))
            w_v = ew_sb[:].rearrange("p (t one) -> p t one", one=1) \
                          .broadcast_to((P, NTT, NTW))
            nc.vector.tensor_tensor(out=oh_v, in0=iota_v, in1=slot_v,
                                    op=mybir.AluOpType.is_equal)
            nc.vector.tensor_tensor(out=oh_v, in0=oh_v, in1=w_v,
                                    op=mybir.AluOpType.mult)

            # ---------- helpers ----------
            UNROLL = 14                       # NT = 98 = 7 iterations x 14

            def prop(table_ap, dest_fm, tag):
                """one propagation: gather+scatter; output lands FEATURE-major
                in dest_fm.  Hardware For_i loop over node tiles; the gathered
                rows are the matmul lhsT (static pool-tile offsets) and the
                one-hot scatter matrices stream as rhs with dynamic offsets,
                producing [F, NTW] feature-major PSUM tiles directly."""
                with nc.named_scope(f"prop_{tag}"):
                    # tiny gpsimd-issued DMA touching the table: executes the
                    # collective-completion wait so the 1-wait-limited dynamic
                    # gathers below don't need it
                    pr = spool.tile([1, 2], dt.bfloat16, tag="pr")
                    nc.gpsimd.dma_start(out=pr[:], in_=table_ap.tensor[0:1, 0:2])
                    with tc.For_i(0, NT, UNROLL) as i0:
                        # stage this iteration's gather indices at a static
                        # SBUF offset (indirect DMA rejects register-offset
                        # index APs)
                        stg = mpool.tile([P, UNROLL * TE], dt.int32, tag="stg")
                        nc.vector.tensor_copy(
                            out=stg[:], in_=src_sb[:, ds(i0 * TE, UNROLL * TE)])
                        for u in range(UNROLL):
                            m_t = mpool.tile([P, TE * F], dt.bfloat16, tag="m")
                            # HW note: indirect DMA honors only ONE offset
                            # column per call, hence one gather per 128-edge
                            # tile.
                            for t in range(TE):
                                nc.gpsimd.indirect_dma_start(
                                    out=m_t[:, t * F:(t + 1) * F],
                                    out_offset=None,
                                    in_=table_ap,
                                    in_offset=bass.IndirectOffsetOnAxis(
                                        ap=stg[:, u * TE + t:u * TE + t + 1], axis=0),
                                )
                            ps = pscat.tile([F, NTW], dt.float32, space="PSUM", tag="ps")
                            for t in range(TE):
                                nc.tensor.matmul(
                                    out=ps[:],
                                    lhsT=m_t[:, t * F:(t + 1) * F],
                                    rhs=oh_sb[:, ds(((i0 + u) * TE + t) * NTW, NTW)],
                                    start=(t == 0),
                                    stop=(t == TE - 1),
                                )
                            nc.vector.tensor_copy(
                                out=dest_fm[:, ds((i0 + u) * NTW, NTW)], in_=ps[:])

            def table_write_and_ag(idx):
                """write s_nm -> bounce[idx] (node-major [SHARD, F]) and allgather."""
                with nc.named_scope(f"ag_{idx}"):
                    bo = bounce[idx]
                    view = bo[:].rearrange("(j p) f -> p j f", p=P)
                    nc.sync.dma_start(out=view, in_=s_nm[:].rearrange("p (j f) -> p j f", f=F))
                    if DBG_NO_AG:
                        for r in range(C):
                            nc.sync.dma_start(out=ag[idx][r * SHARD:(r + 1) * SHARD, :],
                                              in_=bo[:])
                    else:
                        nc.gpsimd.collective_compute(
                            "AllGather",
                            mybir.AluOpType.bypass,
                            replica_groups=rg,
                            ins=[bo.opt()],
                            outs=[ag[idx].opt()],
                        )

            def fm_to_snm(src_t, tag):
                """transpose feature-major tile back into s_nm node-major staging.
                PE transpose streams in_ through the weight port (no register
                offsets), so each slice stages through a fixed tile first."""
                with nc.named_scope(f"nm_{tag}"):
                    with tc.For_i(0, NT // 2, 7) as j0:
                        for u in range(7):
                            stg = spool.tile([F, P], dt.bfloat16, tag="tstg")
                            nc.vector.tensor_copy(
                                out=stg[:], in_=src_t[:, ds((j0 + u) * P, P)])
                            pt = ptr.tile([P, F], dt.bfloat16, space="PSUM", tag="pt")
                            nc.tensor.transpose(out=pt[:], in_=stg[:],
                                                identity=ident[:F, :F])
                            nc.vector.tensor_copy(
                                out=s_nm[:, ds((j0 + u) * F, F)], in_=pt[:])

            def dense(layer, tx0_t, t1_t, s2_t, h_t):
                """h = relu(tx0@W0' + t1@W1 + s2@W2') feature-major, bf16 out."""
                with nc.named_scope(f"dense_{layer}"):
                    wof = layer * 3 * F

                    def chunk(c0, width):
                        pd = pdense.tile([F, NCHUNK], dt.float32, space="PSUM", tag="pd")
                        for ki, rhs_t in enumerate((tx0_t, t1_t, s2_t)):
                            nc.tensor.matmul(
                                out=pd[:, :width],
                                lhsT=w_sb[:, wof + ki * F:wof + (ki + 1) * F],
                                rhs=rhs_t[:, ds(c0, width)],
                                start=(ki == 0),
                                stop=(ki == 2),
                            )
                        nc.scalar.activation(
                            out=h_t[:, ds(c0, width)], in_=pd[:, :width],
                            func=mybir.ActivationFunctionType.Relu,
                            bias=bias_sb[:, layer:layer + 1],
                        )

                    nfull = SHARD // NCHUNK                  # 12 full chunks
                    with tc.For_i(0, nfull * NCHUNK, 2 * NCHUNK) as c0:
                        chunk(c0, NCHUNK)
                        chunk(c0 + NCHUNK, NCHUNK)
                    chunk(nfull * NCHUNK, SHARD - nfull * NCHUNK)

            # ---------- pipeline ----------
            obs_t = res.tile([1, 1], dt.int32)
            nc.gpsimd.tensor_copy(out=obs_t[:], in_=src_sb[0:1, 0:1])

            # initial table: transpose x shard to node-major, allgather
            fm_to_snm(fm["tx0"], "x")
            table_write_and_ag(3)                     # ag[3] = x full

            # Layer 1
            prop(ag[3][:], fm["t1"], "l1a")           # fm t1 = Tx1 own
            fm_to_snm(fm["t1"], "t1")
            table_write_and_ag(0)                     # ag[0] = Tx1 full
            prop(ag[0][:], fm["s2"], "l1b")           # fm s2 = L@Tx1 own
            dense(0, fm["tx0"], fm["t1"], fm["s2"], fm["h"])
            fm_to_snm(fm["h"], "h1")
            table_write_and_ag(1)                     # ag[1] = h1 full

            # Layer 2
            prop(ag[1][:], fm["t1"], "l2a")
            fm_to_snm(fm["t1"], "t1b")
            table_write_and_ag(2)                     # ag[2] = Tx1' full
            prop(ag[2][:], fm["s2"], "l2b")
            dense(1, fm["h"], fm["t1"], fm["s2"], fm["tx0"])   # h2 -> fm["tx0"]

            # final linear [2 x SHARD]
            with nc.named_scope("final"):
                nchunks = (SHARD + NCHUNK - 1) // NCHUNK
                for ci in range(nchunks):
                    c0 = ci * NCHUNK
                    c1 = min(SHARD, c0 + NCHUNK)
                    pf = pdense.tile([2, NCHUNK], dt.float32, space="PSUM", tag="pd")
                    nc.tensor.matmul(out=pf[:, :c1 - c0],
                                     lhsT=w_sb[:, 6 * F:6 * F + 2],
                                     rhs=fm["tx0"][:, c0:c1], start=True, stop=True)
                    ot = spool.tile([2, NCHUNK], dt.bfloat16, tag="ot")
                    nc.scalar.activation(
                        out=ot[:, :c1 - c0], in_=pf[:, :c1 - c0],
                        func=mybir.ActivationFunctionType.Identity,
                        bias=blin_sb[:],
                    )
                    nc.sync.dma_start(out=out_d[:, c0:c1], in_=ot[:, :c1 - c0])

    nc.compile()
    return nc


# --------------------------------------------------------------------------
# input packing
# --------------------------------------------------------------------------
_pre_cache = {}          # edge-structure preprocessing, keyed by content hash


def _preprocess_cached(x, edge_index, edge_weight):
    import hashlib
    ei = np.ascontiguousarray(edge_index)
    ew = np.ascontiguousarray(edge_weight)
    h = hashlib.blake2b(ei.tobytes(), digest_size=16)
    h.update(ew.tobytes())
    key = h.hexdigest()
    if key not in _pre_cache:
        _pre_cache.clear()
        _pre_cache[key] = _preprocess(x, edge_index, edge_weight)
    return _pre_cache[key]


def _pack_inputs(x, edge_index, edge_weight, W1, b1, W2, b2, Wlin, blin):
    new_id, src_idx, slot_a, w_a = _preprocess_cached(x, edge_index, edge_weight)

    xp = np.zeros((NP_PAD, F), np.float32)
    xp[new_id] = x

    # folded dense weights: [W0-W2, W1, 2*W2] per layer, then wlin
    wall = np.concatenate([
        W1[0] - W1[2], W1[1], 2.0 * W1[2],
        W2[0] - W2[2], W2[1], 2.0 * W2[2],
    ], axis=1).astype(BF)                       # [F, 6F]
    wall = np.concatenate([wall, Wlin.astype(BF)], axis=1)  # [F, 6F+2]

    w_bf = w_a.astype(BF)
    slot8 = slot_a.astype(np.uint8)
    xpT = xp.T.astype(BF)                       # [F, NP_PAD]

    in_maps = []
    for c in range(C):
        blob = np.zeros((P, CB), BF)
        blob[0:F, OFF_XT:OFF_XT + SHARD] = xpT[:, c * SHARD:(c + 1) * SHARD]
        # fold stripes: lane q*32+r -> blob[96+r, q*1568 + [w | slot-u8]]
        stripe = np.zeros((P, STRIPE), BF)
        stripe[:, 0:NTT] = w_bf[c]
        stripe[:, NTT:NTT + NTT // 2] = slot8[c].view(np.uint16).view(BF)
        blob[F:P, 0:SHARD] = stripe.reshape(4, 32, STRIPE) \
                                   .transpose(1, 0, 2).reshape(32, SHARD)
        blob[:, OFF_SRC:OFF_SRC + NTT] = src_idx[c].astype(np.uint16).view(BF)
        blob[0:F, OFF_W:OFF_W + 6 * F + 2] = wall
        blob[0:F, OFF_B:OFF_B + 2] = b1.astype(np.float32).view(np.uint16).view(BF).reshape(F, 2)
        blob[0:F, OFF_B + 2:OFF_B + 4] = b2.astype(np.float32).view(np.uint16).view(BF).reshape(F, 2)
        blob[0:2, OFF_B + 4:OFF_B + 6] = blin.astype(np.float32).view(np.uint16).view(BF).reshape(2, 2)

        in_maps.append({"blob": blob})
    return new_id, in_maps


# --------------------------------------------------------------------------
# entry point
# --------------------------------------------------------------------------
def kernel(x, edge_index, edge_weight, W1, b1, W2, b2, Wlin, blin,
           _trace=False, _tmpdir=None):
    global _compiled
    x = np.asarray(x, np.float32)
    W1 = np.asarray(W1, np.float32); W2 = np.asarray(W2, np.float32)
    b1 = np.asarray(b1, np.float32); b2 = np.asarray(b2, np.float32)
    Wlin = np.asarray(Wlin, np.float32); blin = np.asarray(blin, np.float32)

    new_id, in_maps = _pack_inputs(x, edge_index, edge_weight,
                                   W1, b1, W2, b2, Wlin, blin)

    if _compiled is None:
        _compiled = _build_kernel()
    nc = _compiled

    import time as _time
    _t0 = _time.perf_counter()
    try:
        res = run_bass_kernel_spmd(nc, in_maps[:DBG_CORES], core_ids=list(range(DBG_CORES)),
                                   trace=_trace, tmpdir=_tmpdir)
    except ModuleNotFoundError:
        # axon NTFF hook unavailable in this container; run untraced
        res = run_bass_kernel_spmd(nc, in_maps[:DBG_CORES], core_ids=list(range(DBG_CORES)),
                                   trace=False, tmpdir=_tmpdir)
    kernel.last_spmd_wall_s = _time.perf_counter() - _t0

    outs_per_core = [np.asarray(res.results[c]["out"]) for c in range(len(res.results))]
    while len(outs_per_core) < C:
        outs_per_core.append(outs_per_core[-1])
    out_p = np.concatenate(outs_per_core, axis=1)   # [2, NP_PAD]
    out = out_p.T[new_id].astype(np.float32)    # [N, 2]
    if _trace:
        kernel.last_exec_time_ns = res.exec_time_ns
        kernel.last_results = res
    return out



# revision 22
# speedup vs baseline: 1.0899x; 1.0899x over previous
"""Trainium2 Bass kernel for nn_Cheb_35888746725726 (ChebConv K=3 GNN, N=50000,
E=800000, F=H=96, lambda_max=2 -> diag term is 0).

v2 strategy (8 NeuronCores, node/graph-parallel):
 - Host: Chebyshev edge norm, capacity-bounded LPT of nodes into 392 dst-tiles
   of 128 (8 cores x 49), per-edge (pair-idx, dst-slot|parity, weight) planes.
 - Device tables are NODE-major DRAM [25088 pair-rows, 256] bf16 (two 128-col
   padded node rows per 512B row).  One bulk `dma_gather` per ~1024 edges
   fetches pair-rows straight into node-major SBUF tiles [128 edges, 256]; the
   wrong member of each pair is cancelled by a zero weight in the one-hot
   scatter matrices (host bakes parity), so no select/transpose is needed.
 - Scatter: per dst-tile, 2*TE2 accumulating PE matmuls (even/odd candidate
   slices x weighted one-hot [128,128]) -> PSUM [96,128] -> feature-major fm.
 - Tables for the next hop are written by 49 PE transposes of the fm shard
   (staged, For_i) + one strided DMA; AllGather (8 cores) rebuilds the full
   pair-row table between dependent props.
 - Dense 96x96 matmuls run feature-major with host-folded Chebyshev weights:
   out = Tx0 @ (W0-W2) + Tx1 @ W1 + (L@Tx1) @ (2*W2).
"""
import numpy as np
import ml_dtypes

import jax

try:
    jax.config.update("jax_compilation_cache_dir", "/tmp/jax_comp_cache")
    jax.config.update("jax_persistent_cache_min_compile_time_secs", 0.0)
    jax.config.update("jax_persistent_cache_min_entry_size_bytes", 0)
except Exception:
    pass

import concourse.bass as bass
import concourse.bacc as bacc
import concourse.mybir as mybir
import concourse.tile as tile
from concourse.bass import ds
from concourse.bass_utils import run_bass_kernel_spmd
from concourse.masks import make_identity

# ---- problem constants (hardcoded per the harness contract) ----
N = 50000
E = 800000
F = 96
C = 8                    # cores
NP_PAD = 50176           # 8 * 6272
SHARD = NP_PAD // C      # 6272
DT = 49                  # dst tiles per core
DTW = 128                # dst tile width (nodes)
TE2 = 17                 # 128-edge tiles per dst tile (capacity 2176 edges)
ECOLS = DT * TE2         # 833 per-edge plane columns
NPAIR = NP_PAD // 2      # 25088 pair rows
PAIRW = 256              # elements per pair row (2 x 128-col padded nodes)
IDXT = TE2 * DTW // 16   # 136 idx cols per dst tile
IDXCOLS = DT * IDXT      # 6664
P = 128
NCHUNK = 512             # dense matmul node-chunk

# blob column offsets (bf16 [128, CB]); narrow dtypes bitcast into bf16 cols.
# x occupies rows 0:96 of cols 0:SHARD; idx16 hides in rows 96:112 of the same
# cols (part 1) and rows 112:128 cols 0:IDXCOLS-SHARD (part 2).
OFF_W = SHARD                       # [128, ECOLS] per-edge weights bf16
OFF_SP = OFF_W + ECOLS              # [128, ECOLS/2] slot|parity u8 (bitcast)
SPC = (ECOLS + 1) // 2              # 417
OFF_WM = OFF_SP + SPC               # [0:96, 6F+2] folded dense weights
OFF_B = OFF_WM + 6 * F + 2          # [0:96, 6] b1,b2,blin f32 bitcast
CB = OFF_B + 6
IDX2 = IDXCOLS - SHARD              # 392 cols of idx part 2
assert OFF_B % 2 == 0 and CB % 2 == 0

BF = ml_dtypes.bfloat16

_compiled = None


# --------------------------------------------------------------------------
# host-side preprocessing
# --------------------------------------------------------------------------
def _preprocess(x, edge_index, edge_weight):
    src = np.asarray(edge_index[0]).astype(np.int64)
    dst = np.asarray(edge_index[1]).astype(np.int64)
    w = np.asarray(edge_weight).astype(np.float32)

    deg = np.zeros(N, np.float32)
    np.add.at(deg, src, w)
    dis = np.where(deg > 0, 1.0 / np.sqrt(np.maximum(deg, 1e-30)), 0.0).astype(np.float32)
    norm_w = (-dis[src] * w * dis[dst]).astype(np.float32)

    # capacity-bounded LPT: nodes -> 392 tiles of 128, indeg sum <= TE2*128
    indeg = np.bincount(dst, minlength=N).astype(np.int64)
    n_tiles = C * DT
    cap = TE2 * DTW
    order = np.argsort(-indeg, kind="stable")
    import heapq
    heap = [(0, 0, t) for t in range(n_tiles)]
    heapq.heapify(heap)
    tile_assign = np.empty(N, np.int64)
    spill = []
    for n in order:
        placed = False
        while heap:
            load, cnt, t = heapq.heappop(heap)
            if cnt < DTW and load + indeg[n] <= cap:
                tile_assign[n] = t
                heapq.heappush(heap, (load + indeg[n], cnt + 1, t))
                placed = True
                break
            if cnt < DTW:
                spill.append((load, cnt, t))
            # full tiles drop out
        for it in spill:
            heapq.heappush(heap, it)
        spill.clear()
        assert placed, "LPT infeasible: raise TE2"

    order2 = np.argsort(tile_assign, kind="stable")
    slot_in_tile = np.empty(N, np.int64)
    counts = np.bincount(tile_assign, minlength=n_tiles)
    starts = np.concatenate([[0], np.cumsum(counts)[:-1]])
    slot_in_tile[order2] = np.arange(N) - np.repeat(starts, counts)
    new_id = tile_assign * DTW + slot_in_tile

    src_n = new_id[src]
    dst_n = new_id[dst]

    # bucket edges into (core, edge-slot) by destination tile
    o = np.argsort(dst_n, kind="stable")
    es, ed, ew = src_n[o], dst_n[o], norm_w[o]
    gtile = ed // DTW
    tstart = np.searchsorted(gtile, np.arange(n_tiles))
    r = np.arange(E) - tstart[gtile]                  # rank within dst tile
    assert r.max() < TE2 * P, f"tile overflow: {r.max() + 1}"
    core = gtile // DT
    dtile = gtile % DT
    lane = r % P
    etile = r // P                                    # 0..TE2-1
    col = dtile * TE2 + etile

    idxp = np.zeros((C, P, ECOLS), np.int16)          # pair idx per edge slot
    wp = np.zeros((C, P, ECOLS), np.float32)
    sp = np.zeros((C, P, ECOLS), np.uint8)
    idxp[core, lane, col] = (es // 2).astype(np.int16)
    wp[core, lane, col] = ew
    sp[core, lane, col] = (ed - gtile * DTW).astype(np.uint8) | ((es % 2) << 7).astype(np.uint8)

    return new_id, idxp, wp, sp


_pre_cache = {}


def _preprocess_cached(x, edge_index, edge_weight):
    import hashlib
    ei = np.ascontiguousarray(edge_index)
    ew = np.ascontiguousarray(edge_weight)
    h = hashlib.blake2b(ei.tobytes(), digest_size=16)
    h.update(ew.tobytes())
    key = h.hexdigest()
    if key not in _pre_cache:
        _pre_cache.clear()
        _pre_cache[key] = _preprocess(x, edge_index, edge_weight)
    return _pre_cache[key]


# --------------------------------------------------------------------------
# bass kernel builder
# --------------------------------------------------------------------------
def _build_kernel(cfg=()):
    cfg = frozenset(cfg)
    dt = mybir.dt
    nc = bacc.Bacc("TRN2", target_bir_lowering=False, debug=False, num_devices=C)

    blob_d = nc.dram_tensor("blob", [P, CB], dt.bfloat16, kind="ExternalInput")
    out_d = nc.dram_tensor("out", [2, SHARD], dt.bfloat16, kind="ExternalOutput")

    rg = [list(range(C))]
    local_ag = "noag" in cfg
    nrep = 4 if "rep4" in cfg else 1

    with tile.TileContext(nc) as tc:
        with (
            tc.tile_pool(name="res", bufs=1) as res,
            tc.tile_pool(name="mpool", bufs=6) as mpool,      # gather dests
            tc.tile_pool(name="spool", bufs=3) as spool,      # small staging
            tc.tile_pool(name="opool", bufs=2) as opool,      # one-hot planes
            tc.tile_pool(name="pscat", bufs=2, space="PSUM") as pscat,
            tc.tile_pool(name="ptr", bufs=2, space="PSUM") as ptr,
            tc.tile_pool(name="pdense", bufs=2, space="PSUM") as pdense,
            tc.tile_pool(name="dram", bufs=1, space="DRAM") as dram,
        ):
            # ---------- resident loads ----------
            fm = {
                "tx0": res.tile([F, SHARD], dt.bfloat16, name="fm_tx0"),
                "t1": res.tile([F, SHARD], dt.bfloat16, name="fm_t1"),
                "s2": res.tile([F, SHARD], dt.bfloat16, name="fm_s2"),
                "h": res.tile([F, SHARD], dt.bfloat16, name="fm_h"),
            }
            nc.sync.dma_start(out=fm["tx0"][:], in_=blob_d[0:F, 0:SHARD])

            idx_sb = res.tile([P, IDXCOLS], dt.int16)
            nc.sync.dma_start(out=idx_sb[0:16, 0:SHARD],
                              in_=blob_d[F:F + 16, 0:SHARD].bitcast(dt.int16))
            nc.sync.dma_start(out=idx_sb[0:16, SHARD:IDXCOLS],
                              in_=blob_d[F + 16:F + 32, 0:IDX2].bitcast(dt.int16))
            for g in range(1, 8):       # replicate idx into all 16-part groups
                nc.sync.dma_start(out=idx_sb[16 * g:16 * (g + 1), :],
                                  in_=idx_sb[0:16, :])

            w_edge = res.tile([P, ECOLS], dt.bfloat16)
            nc.sync.dma_start(out=w_edge[:], in_=blob_d[:, OFF_W:OFF_W + ECOLS])
            sp8 = res.tile([P, 2 * SPC], dt.uint8)
            nc.sync.dma_start(out=sp8[:], in_=blob_d[:, OFF_SP:OFF_SP + SPC].bitcast(dt.uint8))

            # slot (bf16) and parity masks (bf16) from slot|par<<7
            slot_sb = res.tile([P, ECOLS], dt.bfloat16)
            par_sb = res.tile([P, ECOLS], dt.bfloat16)
            spi = res.tile([P, ECOLS], dt.int32)
            nc.vector.tensor_copy(out=spi[:], in_=sp8[:, 0:ECOLS])
            pari = res.tile([P, ECOLS], dt.int32)
            nc.vector.tensor_scalar(out=pari[:], in0=spi[:], scalar1=7, scalar2=None,
                                    op0=mybir.AluOpType.logical_shift_right)
            nc.vector.tensor_copy(out=par_sb[:], in_=pari[:])
            nc.vector.tensor_scalar(out=spi[:], in0=spi[:], scalar1=127, scalar2=None,
                                    op0=mybir.AluOpType.bitwise_and)
            nc.vector.tensor_copy(out=slot_sb[:], in_=spi[:])
            # per-candidate weights: even = w*(1-par), odd = w*par
            w_ev = res.tile([P, ECOLS], dt.bfloat16)
            w_od = res.tile([P, ECOLS], dt.bfloat16)
            nc.vector.tensor_tensor(out=w_od[:], in0=w_edge[:], in1=par_sb[:],
                                    op=mybir.AluOpType.mult)
            nc.vector.tensor_tensor(out=w_ev[:], in0=w_edge[:], in1=w_od[:],
                                    op=mybir.AluOpType.subtract)

            w_sb = res.tile([F, 6 * F + 2], dt.bfloat16)
            nc.sync.dma_start(out=w_sb[:], in_=blob_d[0:F, OFF_WM:OFF_WM + 6 * F + 2])
            bias_sb = res.tile([F, 2], dt.float32)
            nc.sync.dma_start(out=bias_sb[:],
                              in_=blob_d[0:F, OFF_B:OFF_B + 4].bitcast(dt.float32))
            blin_sb = res.tile([2, 1], dt.float32)
            nc.sync.dma_start(out=blin_sb[:],
                              in_=blob_d[0:2, OFF_B + 4:OFF_B + 6].bitcast(dt.float32))
            ident = res.tile([P, P], dt.bfloat16)
            make_identity(nc, ident[:])
            iota_i = res.tile([P, DTW], dt.int32)
            nc.gpsimd.iota(iota_i[:], pattern=[[1, DTW]], base=0, channel_multiplier=0)
            iota_b = res.tile([P, DTW], dt.bfloat16)
            nc.vector.tensor_copy(out=iota_b[:], in_=iota_i[:])

            # node-major staging for table writes
            s_nm = res.tile([P, DT * F], dt.bfloat16)

            # internal DRAM tables: [NP_PAD, 128] node-major, 128-col padded
            # rows; dma_gather views the same memory as [NPAIR, 256] pair rows
            bounce = [dram.tile([SHARD, PAIRW // 2], dt.bfloat16, name=f"bounce{i}")
                      for i in range(4 * nrep)]
            ag = [dram.tile([NP_PAD, PAIRW // 2], dt.bfloat16,
                            addr_space=("Local" if local_ag else "Shared"), name=f"ag{i}")
                  for i in range(4 * nrep)]

            # ---------- helpers ----------
            def fm_to_table(src_t, idx, tag):
                """PE-transpose fm shard into node-major staging, DMA to
                bounce[idx] (pair rows), allgather into ag[idx]."""
                with nc.named_scope(f"tw_{tag}"):
                    with tc.For_i(0, DT, 7) as j0:
                        for u in range(7):
                            stg = spool.tile([F, P], dt.bfloat16, tag="tstg")
                            nc.vector.tensor_copy(
                                out=stg[:], in_=src_t[0:F, ds((j0 + u) * P, P)])
                            pt = ptr.tile([P, F], dt.bfloat16, space="PSUM", tag="pt")
                            nc.tensor.transpose(out=pt[:], in_=stg[:],
                                                identity=ident[:F, :F])
                            nc.vector.tensor_copy(
                                out=s_nm[:, ds((j0 + u) * F, F)], in_=pt[:])
                    bo = bounce[idx]
                    nc.sync.dma_start(
                        out=bo[:].rearrange("(j p) f -> p j f", p=P)[:, :, 0:F],
                        in_=s_nm[:].rearrange("p (j f) -> p j f", f=F))
                    if local_ag:
                        for rr in range(C):
                            nc.sync.dma_start(
                                out=ag[idx][rr * SHARD:(rr + 1) * SHARD, :],
                                in_=bo[:])
                    else:
                        nc.gpsimd.collective_compute(
                            "AllGather", mybir.AluOpType.bypass,
                            replica_groups=rg, ins=[bo.opt()], outs=[ag[idx].opt()])

            GCALLS = ((1024, 8), (1024, 8), (128, 1))     # (idx per call, tiles)

            def prop_tile(table_ap, dest_fm, d_expr):
                """one dst-tile: 3 dma_gathers + one-hot build + 34 matmuls."""
                # stage idx/slot/w slices at static offsets (reg-offset APs are
                # not accepted by the custom DMA / PE weight port)
                stg_i = spool.tile([P, IDXT], dt.int16, tag="stgi")
                nc.vector.tensor_copy(out=stg_i[:], in_=idx_sb[:, ds(d_expr * IDXT, IDXT)])
                stg_s = spool.tile([P, 3 * TE2], dt.bfloat16, tag="stgs")
                nc.vector.tensor_copy(out=stg_s[:, 0:TE2],
                                      in_=slot_sb[:, ds(d_expr * TE2, TE2)])
                nc.vector.tensor_copy(out=stg_s[:, TE2:2 * TE2],
                                      in_=w_ev[:, ds(d_expr * TE2, TE2)])
                nc.vector.tensor_copy(out=stg_s[:, 2 * TE2:3 * TE2],
                                      in_=w_od[:, ds(d_expr * TE2, TE2)])

                ms = []
                icol = 0
                for nidx, ntile in GCALLS:
                    m_t = mpool.tile([P, ntile * PAIRW], dt.bfloat16, tag=f"m{ntile}")
                    nc.gpsimd.dma_gather(
                        out_ap=m_t[:].rearrange("p (t f) -> p t f", f=PAIRW),
                        in_ap=table_ap,
                        idxs_ap=stg_i[:, icol:icol + nidx // 16],
                        num_idxs=nidx,
                        num_idxs_reg=nidx,
                        elem_size=PAIRW,
                        single_packet=True,
                    )
                    icol += nidx // 16
                    ms.append((m_t, ntile))

                # weighted one-hot [128, TE2, 256]: cols 0:128 even cand,
                # 128:256 odd cand
                oh = opool.tile([P, TE2 * 2 * DTW], dt.bfloat16, tag="oh")
                ohv = oh[:].rearrange("p (t h s) -> p t h s", h=2, s=DTW)
                iota_v = iota_b[:].rearrange("p (o1 o2 s) -> p o1 o2 s", o1=1, o2=1) \
                                  .broadcast_to((P, TE2, 2, DTW))
                slot_v = stg_s[:, 0:TE2].rearrange("p (t o1 o2) -> p t o1 o2", o1=1, o2=1) \
                                        .broadcast_to((P, TE2, 2, DTW))
                nc.vector.tensor_tensor(out=ohv, in0=iota_v, in1=slot_v,
                                        op=mybir.AluOpType.is_equal)
                wev_v = stg_s[:, TE2:2 * TE2].rearrange("p (t o) -> p t o", o=1) \
                                             .broadcast_to((P, TE2, DTW))
                wod_v = stg_s[:, 2 * TE2:3 * TE2].rearrange("p (t o) -> p t o", o=1) \
                                                 .broadcast_to((P, TE2, DTW))
                nc.vector.tensor_tensor(out=ohv[:, :, 0, :], in0=ohv[:, :, 0, :],
                                        in1=wev_v, op=mybir.AluOpType.mult)
                nc.vector.tensor_tensor(out=ohv[:, :, 1, :], in0=ohv[:, :, 1, :],
                                        in1=wod_v, op=mybir.AluOpType.mult)

                ps = pscat.tile([F, DTW], dt.float32, space="PSUM", tag="ps")
                t = 0
                for m_t, ntile in ms:
                    for tt in range(ntile):
                        mv = m_t[:].rearrange("p (t f) -> p t f", f=PAIRW)
                        for h in range(2):
                            nc.tensor.matmul(
                                out=ps[:],
                                lhsT=mv[:, tt, h * P:h * P + F],
                                rhs=ohv[:, t, h, :],
                                start=(t == 0 and h == 0),
                                stop=(t == TE2 - 1 and h == 1),
                            )
                        t += 1
                nc.vector.tensor_copy(out=dest_fm[0:F, ds(d_expr * DTW, DTW)], in_=ps[:])

            def prop(table_t, dest_fm, tag):
                if "noprop" in cfg:
                    nc.vector.memset(dest_fm[:], 0)
                    return
                with nc.named_scope(f"prop_{tag}"):
                    # pair-row view [NPAIR, 256] of the node-major table
                    table_ap = table_t[:].rearrange("(n two) f -> n (two f)", two=2)
                    # gpsimd pre-touch executes the collective-completion wait
                    pr = spool.tile([1, 2], dt.bfloat16, tag="pr")
                    nc.gpsimd.dma_start(out=pr[:], in_=table_t[0:1, 0:2])
                    with tc.For_i(0, DT - 1, 2) as d0:
                        prop_tile(table_ap, dest_fm, d0)
                        prop_tile(table_ap, dest_fm, d0 + 1)
                    prop_tile(table_ap, dest_fm, DT - 1)

            def dense(layer, tx0_t, t1_t, s2_t, h_t):
                """h = relu(tx0@W0' + t1@W1 + s2@W2') feature-major, bf16."""
                with nc.named_scope(f"dense_{layer}"):
                    wof = layer * 3 * F

                    def chunk(c0, width):
                        pd = pdense.tile([F, NCHUNK], dt.float32, space="PSUM", tag="pd")
                        for ki, rhs_t in enumerate((tx0_t, t1_t, s2_t)):
                            nc.tensor.matmul(
                                out=pd[:, :width],
                                lhsT=w_sb[:, wof + ki * F:wof + (ki + 1) * F],
                                rhs=rhs_t[0:F, ds(c0, width)],
                                start=(ki == 0),
                                stop=(ki == 2),
                            )
                        nc.scalar.activation(
                            out=h_t[0:F, ds(c0, width)], in_=pd[:, :width],
                            func=mybir.ActivationFunctionType.Relu,
                            bias=bias_sb[:, layer:layer + 1],
                        )

                    nfull = SHARD // NCHUNK
                    with tc.For_i(0, nfull * NCHUNK, 2 * NCHUNK) as c0:
                        chunk(c0, NCHUNK)
                        chunk(c0 + NCHUNK, NCHUNK)
                    chunk(nfull * NCHUNK, SHARD - nfull * NCHUNK)

            # ---------- pipeline ----------
            for _r in range(nrep):
                rb = 4 * _r
                fm_to_table(fm["tx0"], rb + 3, "x")       # ag[3] = x table

                prop(ag[rb + 3], fm["t1"], "l1a")         # t1 = L @ x
                fm_to_table(fm["t1"], rb + 0, "t1")
                prop(ag[rb + 0], fm["s2"], "l1b")         # s2 = L @ t1
                dense(0, fm["tx0"], fm["t1"], fm["s2"], fm["h"])
                fm_to_table(fm["h"], rb + 1, "h1")

                prop(ag[rb + 1], fm["t1"], "l2a")
                fm_to_table(fm["t1"], rb + 2, "t1b")
                prop(ag[rb + 2], fm["s2"], "l2b")
                dense(1, fm["h"], fm["t1"], fm["s2"],
                      fm["tx0" if nrep == 1 else "h"])

            hfin = fm["tx0" if nrep == 1 else "h"]
            with nc.named_scope("final"):
                nchunks = (SHARD + NCHUNK - 1) // NCHUNK
                for ci in range(nchunks):
                    c0 = ci * NCHUNK
                    c1 = min(SHARD, c0 + NCHUNK)
                    pf = pdense.tile([2, NCHUNK], dt.float32, space="PSUM", tag="pd")
                    nc.tensor.matmul(out=pf[:, :c1 - c0],
                                     lhsT=w_sb[:, 6 * F:6 * F + 2],
                                     rhs=hfin[0:F, c0:c1], start=True, stop=True)
                    ot = spool.tile([2, NCHUNK], dt.bfloat16, tag="ot")
                    nc.scalar.activation(
                        out=ot[:, :c1 - c0], in_=pf[:, :c1 - c0],
                        func=mybir.ActivationFunctionType.Identity,
                        bias=blin_sb[:],
                    )
                    nc.sync.dma_start(out=out_d[:, c0:c1], in_=ot[:, :c1 - c0])

    nc.compile()
    # memoize the BIR json: bass2jax re-serializes it on every call otherwise
    _json = nc.to_json_bytes()
    nc.to_json_bytes = lambda: _json
    return nc


# --------------------------------------------------------------------------
# input packing
# --------------------------------------------------------------------------
def _pack_inputs(x, edge_index, edge_weight, W1, b1, W2, b2, Wlin, blin):
    new_id, idxp, wp, sp = _preprocess_cached(x, edge_index, edge_weight)

    xp = np.zeros((NP_PAD, F), np.float32)
    xp[new_id] = x
    xpT = xp.T.astype(BF)

    wall = np.concatenate([
        W1[0] - W1[2], W1[1], 2.0 * W1[2],
        W2[0] - W2[2], W2[1], 2.0 * W2[2],
    ], axis=1).astype(BF)
    wall = np.concatenate([wall, Wlin.astype(BF)], axis=1)

    # idx wrapped for dma_gather: per dst-tile block of IDXT cols; idx j of the
    # tile's 2176-list sits at (j%16, block + j//16)
    in_maps = []
    for c in range(C):
        idxw = idxp[c].reshape(P, DT, TE2).transpose(1, 2, 0).reshape(DT, IDXT, 16)
        idxw = idxw.transpose(0, 2, 1).reshape(DT * 16, IDXT)  # [(D,16p), IDXT]
        idx_plane = np.zeros((16, IDXCOLS), np.int16)
        for d in range(DT):
            idx_plane[:, d * IDXT:(d + 1) * IDXT] = idxw[d * 16:(d + 1) * 16, :]

        blob = np.zeros((P, CB), BF)
        blob[0:F, 0:SHARD] = xpT[:, c * SHARD:(c + 1) * SHARD]
        blob[F:F + 16, 0:SHARD] = idx_plane[:, 0:SHARD].view(BF)
        blob[F + 16:F + 32, 0:IDX2] = idx_plane[:, SHARD:IDXCOLS].view(BF)
        blob[:, OFF_W:OFF_W + ECOLS] = wp[c].astype(BF)
        spc = np.zeros((P, SPC * 2), np.uint8)
        spc[:, 0:ECOLS] = sp[c]
        blob[:, OFF_SP:OFF_SP + SPC] = spc.view(np.uint16).view(BF)
        blob[0:F, OFF_WM:OFF_WM + 6 * F + 2] = wall
        blob[0:F, OFF_B:OFF_B + 2] = b1.astype(np.float32).view(np.uint16).view(BF).reshape(F, 2)
        blob[0:F, OFF_B + 2:OFF_B + 4] = b2.astype(np.float32).view(np.uint16).view(BF).reshape(F, 2)
        blob[0:2, OFF_B + 4:OFF_B + 6] = blin.astype(np.float32).view(np.uint16).view(BF).reshape(2, 2)
        in_maps.append({"blob": blob})
    return new_id, in_maps


# --------------------------------------------------------------------------
# entry point
# --------------------------------------------------------------------------
def kernel(x, edge_index, edge_weight, W1, b1, W2, b2, Wlin, blin,
           _trace=False, _tmpdir=None):
    global _compiled
    x = np.asarray(x, np.float32)
    W1 = np.asarray(W1, np.float32); W2 = np.asarray(W2, np.float32)
    b1 = np.asarray(b1, np.float32); b2 = np.asarray(b2, np.float32)
    Wlin = np.asarray(Wlin, np.float32); blin = np.asarray(blin, np.float32)

    new_id, in_maps = _pack_inputs(x, edge_index, edge_weight,
                                   W1, b1, W2, b2, Wlin, blin)

    if _compiled is None:
        _compiled = _build_kernel()
    nc = _compiled

    import time as _time
    _t0 = _time.perf_counter()
    try:
        res = run_bass_kernel_spmd(nc, in_maps, core_ids=list(range(C)),
                                   trace=_trace, tmpdir=_tmpdir)
    except ModuleNotFoundError:
        res = run_bass_kernel_spmd(nc, in_maps, core_ids=list(range(C)),
                                   trace=False, tmpdir=_tmpdir)
    kernel.last_spmd_wall_s = _time.perf_counter() - _t0

    outs_per_core = [np.asarray(res.results[c]["out"]) for c in range(len(res.results))]
    out_p = np.concatenate(outs_per_core, axis=1)   # [2, NP_PAD]
    out = out_p.T[new_id].astype(np.float32)
    if _trace:
        kernel.last_exec_time_ns = res.exec_time_ns
        kernel.last_results = res
    return out


# revision 28
# speedup vs baseline: 1.5021x; 1.3782x over previous
"""Trainium2 Bass kernel for nn_Cheb_35888746725726 (ChebConv K=3 GNN, N=50000,
E=800000, F=H=96, lambda_max=2 -> diag term is 0).

v2 strategy (8 NeuronCores, node/graph-parallel):
 - Host: Chebyshev edge norm, capacity-bounded LPT of nodes into 392 dst-tiles
   of 128 (8 cores x 49), per-edge (pair-idx, dst-slot|parity, weight) planes.
 - Device tables are NODE-major DRAM [25088 pair-rows, 256] bf16 (two 128-col
   padded node rows per 512B row).  One bulk `dma_gather` per ~1024 edges
   fetches pair-rows straight into node-major SBUF tiles [128 edges, 256]; the
   wrong member of each pair is cancelled by a zero weight in the one-hot
   scatter matrices (host bakes parity), so no select/transpose is needed.
 - Scatter: per dst-tile, 2*TE2 accumulating PE matmuls (even/odd candidate
   slices x weighted one-hot [128,128]) -> PSUM [96,128] -> feature-major fm.
 - Tables for the next hop are written by 49 PE transposes of the fm shard
   (staged, For_i) + one strided DMA; AllGather (8 cores) rebuilds the full
   pair-row table between dependent props.
 - Dense 96x96 matmuls run feature-major with host-folded Chebyshev weights:
   out = Tx0 @ (W0-W2) + Tx1 @ W1 + (L@Tx1) @ (2*W2).
"""
import numpy as np
import ml_dtypes

import jax

try:
    jax.config.update("jax_compilation_cache_dir", "/tmp/jax_comp_cache")
    jax.config.update("jax_persistent_cache_min_compile_time_secs", 0.0)
    jax.config.update("jax_persistent_cache_min_entry_size_bytes", 0)
except Exception:
    pass

import concourse.bass as bass
import concourse.bacc as bacc
import concourse.mybir as mybir
import concourse.tile as tile
from concourse.bass import ds
from concourse.bass_utils import run_bass_kernel_spmd
from concourse.masks import make_identity

# ---- problem constants (hardcoded per the harness contract) ----
N = 50000
E = 800000
F = 96
C = 8                    # cores
NP_PAD = 50176           # 8 * 6272
SHARD = NP_PAD // C      # 6272
DT = 49                  # dst tiles per core
DTW = 128                # dst tile width (nodes)
TE2 = 17                 # 128-edge tiles per dst tile (capacity 2176 edges)
ECOLS = DT * TE2         # 833 per-edge plane columns
NPAIR = NP_PAD // 2      # 25088 pair rows
PAIRW = 256              # elements per pair row (2 x 128-col padded nodes)
IDXT = TE2 * DTW // 16   # 136 idx cols per dst tile
IDXCOLS = DT * IDXT      # 6664
P = 128
NCHUNK = 512             # dense matmul node-chunk

# blob column offsets (bf16 [128, CB]); narrow dtypes bitcast into bf16 cols.
# x is int8 (scale folded into layer-1 weights) in rows 0:96 of cols 0:XC;
# idx16 hides in rows 96:128 of the same cols (2 groups) + a remainder strip.
XC = SHARD // 2                     # 3136 bf16 cols holding int8 x
SPC = (ECOLS + 1) // 2              # 417
OFF_W = XC                          # [128, SPC] edge |w| u8 (bitcast)
OFF_SP = OFF_W + SPC                # [128, SPC] slot|parity u8 (bitcast)
OFF_I2 = OFF_SP + SPC               # [0:16, IDX2] idx remainder (int16 bitcast)
IDX2 = IDXCOLS - 2 * XC             # 392
OFF_WM = OFF_I2 + IDX2              # [0:96, 6F+2] folded dense weights
OFF_B = OFF_WM + 6 * F + 2          # [0:96, 6] b1,b2,blin f32 bitcast
CB = OFF_B + 6
assert OFF_B % 2 == 0 and CB % 2 == 0

BF = ml_dtypes.bfloat16

_compiled = None


# --------------------------------------------------------------------------
# host-side preprocessing
# --------------------------------------------------------------------------
def _preprocess(x, edge_index, edge_weight):
    src = np.asarray(edge_index[0]).astype(np.int64)
    dst = np.asarray(edge_index[1]).astype(np.int64)
    w = np.asarray(edge_weight).astype(np.float32)

    deg = np.zeros(N, np.float32)
    np.add.at(deg, src, w)
    dis = np.where(deg > 0, 1.0 / np.sqrt(np.maximum(deg, 1e-30)), 0.0).astype(np.float32)
    norm_w = (-dis[src] * w * dis[dst]).astype(np.float32)

    # capacity-bounded LPT: nodes -> 392 tiles of 128, indeg sum <= TE2*128
    indeg = np.bincount(dst, minlength=N).astype(np.int64)
    n_tiles = C * DT
    cap = TE2 * DTW
    order = np.argsort(-indeg, kind="stable")
    import heapq
    heap = [(0, 0, t) for t in range(n_tiles)]
    heapq.heapify(heap)
    tile_assign = np.empty(N, np.int64)
    spill = []
    for n in order:
        placed = False
        while heap:
            load, cnt, t = heapq.heappop(heap)
            if cnt < DTW and load + indeg[n] <= cap:
                tile_assign[n] = t
                heapq.heappush(heap, (load + indeg[n], cnt + 1, t))
                placed = True
                break
            if cnt < DTW:
                spill.append((load, cnt, t))
            # full tiles drop out
        for it in spill:
            heapq.heappush(heap, it)
        spill.clear()
        assert placed, "LPT infeasible: raise TE2"

    order2 = np.argsort(tile_assign, kind="stable")
    slot_in_tile = np.empty(N, np.int64)
    counts = np.bincount(tile_assign, minlength=n_tiles)
    starts = np.concatenate([[0], np.cumsum(counts)[:-1]])
    slot_in_tile[order2] = np.arange(N) - np.repeat(starts, counts)
    new_id = tile_assign * DTW + slot_in_tile

    src_n = new_id[src]
    dst_n = new_id[dst]

    # bucket edges into (core, edge-slot) by destination tile
    o = np.argsort(dst_n, kind="stable")
    es, ed, ew = src_n[o], dst_n[o], norm_w[o]
    gtile = ed // DTW
    tstart = np.searchsorted(gtile, np.arange(n_tiles))
    r = np.arange(E) - tstart[gtile]                  # rank within dst tile
    assert r.max() < TE2 * P, f"tile overflow: {r.max() + 1}"
    core = gtile // DT
    dtile = gtile % DT
    lane = r % P
    etile = r // P                                    # 0..TE2-1
    col = dtile * TE2 + etile

    idxp = np.zeros((C, P, ECOLS), np.int16)          # pair idx per edge slot
    wp = np.zeros((C, P, ECOLS), np.float32)
    sp = np.zeros((C, P, ECOLS), np.uint8)
    idxp[core, lane, col] = (es // 2).astype(np.int16)
    wp[core, lane, col] = ew
    sp[core, lane, col] = (ed - gtile * DTW).astype(np.uint8) | ((es % 2) << 7).astype(np.uint8)

    return new_id, idxp, wp, sp


_pre_cache = {}


def _preprocess_cached(x, edge_index, edge_weight):
    import hashlib
    ei = np.ascontiguousarray(edge_index)
    ew = np.ascontiguousarray(edge_weight)
    h = hashlib.blake2b(ei.tobytes(), digest_size=16)
    h.update(ew.tobytes())
    key = h.hexdigest()
    if key not in _pre_cache:
        _pre_cache.clear()
        _pre_cache[key] = _preprocess(x, edge_index, edge_weight)
    return _pre_cache[key]


# --------------------------------------------------------------------------
# bass kernel builder
# --------------------------------------------------------------------------
def _build_kernel(cfg=()):
    cfg = frozenset(cfg)
    dt = mybir.dt
    nc = bacc.Bacc("TRN2", target_bir_lowering=False, debug=False, num_devices=C)

    blob_d = nc.dram_tensor("blob", [P, CB], dt.bfloat16, kind="ExternalInput")
    out_d = nc.dram_tensor("out", [2, SHARD], dt.bfloat16, kind="ExternalOutput")

    rg = [list(range(C))]
    local_ag = "noag" in cfg
    nrep = 4 if "rep4" in cfg else 1

    with tile.TileContext(nc) as tc:
        with (
            tc.tile_pool(name="res", bufs=1) as res,
            tc.tile_pool(name="mpool", bufs=6) as mpool,      # gather dests
            tc.tile_pool(name="spool", bufs=3) as spool,      # small staging
            tc.tile_pool(name="opool", bufs=2) as opool,      # one-hot planes
            tc.tile_pool(name="pscat", bufs=2, space="PSUM") as pscat,
            tc.tile_pool(name="ptr", bufs=2, space="PSUM") as ptr,
            tc.tile_pool(name="pdense", bufs=2, space="PSUM") as pdense,
            tc.tile_pool(name="dram", bufs=1, space="DRAM") as dram,
        ):
            # ---------- resident loads ----------
            fm = {
                "tx0": res.tile([F, SHARD], dt.bfloat16, name="fm_tx0"),
                "t1": res.tile([F, SHARD], dt.bfloat16, name="fm_t1"),
                "s2": res.tile([F, SHARD], dt.bfloat16, name="fm_s2"),
                "h": res.tile([F, SHARD], dt.bfloat16, name="fm_h"),
            }
            x8 = res.tile([F, SHARD], dt.int8)
            nc.sync.dma_start(out=x8[:], in_=blob_d[0:F, 0:XC].bitcast(dt.int8))
            nc.vector.tensor_copy(out=fm["tx0"][:], in_=x8[:])

            idx_sb = res.tile([P, IDXCOLS], dt.int16)
            nc.sync.dma_start(out=idx_sb[0:16, 0:XC],
                              in_=blob_d[F:F + 16, 0:XC].bitcast(dt.int16))
            nc.sync.dma_start(out=idx_sb[0:16, XC:2 * XC],
                              in_=blob_d[F + 16:F + 32, 0:XC].bitcast(dt.int16))
            nc.sync.dma_start(out=idx_sb[0:16, 2 * XC:IDXCOLS],
                              in_=blob_d[0:16, OFF_I2:OFF_I2 + IDX2].bitcast(dt.int16))
            for g in range(1, 8):       # replicate idx into all 16-part groups
                nc.sync.dma_start(out=idx_sb[16 * g:16 * (g + 1), :],
                                  in_=idx_sb[0:16, :])

            w8 = res.tile([P, 2 * SPC], dt.uint8)
            nc.sync.dma_start(out=w8[:], in_=blob_d[:, OFF_W:OFF_W + SPC].bitcast(dt.uint8))
            w_edge = res.tile([P, ECOLS], dt.bfloat16)
            nc.vector.tensor_copy(out=w_edge[:], in_=w8[:, 0:ECOLS])
            sp8 = res.tile([P, 2 * SPC], dt.uint8)
            nc.sync.dma_start(out=sp8[:], in_=blob_d[:, OFF_SP:OFF_SP + SPC].bitcast(dt.uint8))

            # slot (bf16) and parity masks (bf16) from slot|par<<7
            slot_sb = res.tile([P, ECOLS], dt.bfloat16)
            par_sb = res.tile([P, ECOLS], dt.bfloat16)
            spi = res.tile([P, ECOLS], dt.int32)
            nc.vector.tensor_copy(out=spi[:], in_=sp8[:, 0:ECOLS])
            pari = res.tile([P, ECOLS], dt.int32)
            nc.vector.tensor_scalar(out=pari[:], in0=spi[:], scalar1=7, scalar2=None,
                                    op0=mybir.AluOpType.logical_shift_right)
            nc.vector.tensor_copy(out=par_sb[:], in_=pari[:])
            nc.vector.tensor_scalar(out=spi[:], in0=spi[:], scalar1=127, scalar2=None,
                                    op0=mybir.AluOpType.bitwise_and)
            nc.vector.tensor_copy(out=slot_sb[:], in_=spi[:])
            # per-candidate weights: even = w*(1-par), odd = w*par
            w_ev = res.tile([P, ECOLS], dt.bfloat16)
            w_od = res.tile([P, ECOLS], dt.bfloat16)
            nc.vector.tensor_tensor(out=w_od[:], in0=w_edge[:], in1=par_sb[:],
                                    op=mybir.AluOpType.mult)
            nc.vector.tensor_tensor(out=w_ev[:], in0=w_edge[:], in1=w_od[:],
                                    op=mybir.AluOpType.subtract)

            w_sb = res.tile([F, 6 * F + 2], dt.bfloat16)
            nc.sync.dma_start(out=w_sb[:], in_=blob_d[0:F, OFF_WM:OFF_WM + 6 * F + 2])
            bias_sb = res.tile([F, 2], dt.float32)
            nc.sync.dma_start(out=bias_sb[:],
                              in_=blob_d[0:F, OFF_B:OFF_B + 4].bitcast(dt.float32))
            blin_sb = res.tile([2, 1], dt.float32)
            nc.sync.dma_start(out=blin_sb[:],
                              in_=blob_d[0:2, OFF_B + 4:OFF_B + 6].bitcast(dt.float32))
            ident = res.tile([P, P], dt.bfloat16)
            make_identity(nc, ident[:])
            iota_i = res.tile([P, DTW], dt.int32)
            nc.gpsimd.iota(iota_i[:], pattern=[[1, DTW]], base=0, channel_multiplier=0)
            iota_b = res.tile([P, DTW], dt.bfloat16)
            nc.vector.tensor_copy(out=iota_b[:], in_=iota_i[:])

            # node-major staging for table writes
            s_nm = res.tile([P, DT * F], dt.bfloat16)

            # internal DRAM tables: [NP_PAD, 128] node-major, 128-col padded
            # rows; dma_gather views the same memory as [NPAIR, 256] pair rows
            bounce = [dram.tile([SHARD, PAIRW // 2], dt.bfloat16, name=f"bounce{i}")
                      for i in range(4 * nrep)]
            ag = [dram.tile([NP_PAD, PAIRW // 2], dt.bfloat16,
                            addr_space=("Local" if local_ag else "Shared"), name=f"ag{i}")
                  for i in range(4 * nrep)]

            # ---------- helpers ----------
            def fm_to_table(src_t, idx, tag):
                """PE-transpose fm shard into node-major staging, DMA to
                bounce[idx] (pair rows), allgather into ag[idx]."""
                with nc.named_scope(f"tw_{tag}"):
                    with tc.For_i(0, DT, 7) as j0:
                        for u in range(7):
                            stg = spool.tile([F, P], dt.bfloat16, tag="tstg")
                            nc.vector.tensor_copy(
                                out=stg[:], in_=src_t[0:F, ds((j0 + u) * P, P)])
                            pt = ptr.tile([P, F], dt.bfloat16, space="PSUM", tag="pt")
                            nc.tensor.transpose(out=pt[:], in_=stg[:],
                                                identity=ident[:F, :F])
                            nc.vector.tensor_copy(
                                out=s_nm[:, ds((j0 + u) * F, F)], in_=pt[:])
                    bo = bounce[idx]
                    nc.sync.dma_start(
                        out=bo[:].rearrange("(j p) f -> p j f", p=P)[:, :, 0:F],
                        in_=s_nm[:].rearrange("p (j f) -> p j f", f=F))
                    if local_ag:
                        for rr in range(C):
                            nc.sync.dma_start(
                                out=ag[idx][rr * SHARD:(rr + 1) * SHARD, :],
                                in_=bo[:])
                    else:
                        nc.gpsimd.collective_compute(
                            "AllGather", mybir.AluOpType.bypass,
                            replica_groups=rg, ins=[bo.opt()], outs=[ag[idx].opt()])

            GCALLS = ((1024, 8), (1024, 8), (128, 1))     # (idx per call, tiles)

            def prop_tile(table_ap, dest_fm, d_expr):
                """one dst-tile: 3 dma_gathers + one-hot build + 34 matmuls."""
                # stage idx/slot/w slices at static offsets (reg-offset APs are
                # not accepted by the custom DMA / PE weight port)
                stg_i = spool.tile([P, IDXT], dt.int16, tag="stgi")
                nc.vector.tensor_copy(out=stg_i[:], in_=idx_sb[:, ds(d_expr * IDXT, IDXT)])
                stg_s = spool.tile([P, 3 * TE2], dt.bfloat16, tag="stgs")
                nc.vector.tensor_copy(out=stg_s[:, 0:TE2],
                                      in_=slot_sb[:, ds(d_expr * TE2, TE2)])
                nc.vector.tensor_copy(out=stg_s[:, TE2:2 * TE2],
                                      in_=w_ev[:, ds(d_expr * TE2, TE2)])
                nc.vector.tensor_copy(out=stg_s[:, 2 * TE2:3 * TE2],
                                      in_=w_od[:, ds(d_expr * TE2, TE2)])

                ms = []
                icol = 0
                for nidx, ntile in GCALLS:
                    m_t = mpool.tile([P, ntile * PAIRW], dt.bfloat16, tag=f"m{ntile}")
                    nc.gpsimd.dma_gather(
                        out_ap=m_t[:].rearrange("p (t f) -> p t f", f=PAIRW),
                        in_ap=table_ap,
                        idxs_ap=stg_i[:, icol:icol + nidx // 16],
                        num_idxs=nidx,
                        num_idxs_reg=nidx,
                        elem_size=PAIRW,
                        single_packet=True,
                    )
                    icol += nidx // 16
                    ms.append((m_t, ntile))

                # weighted one-hot [128, TE2, 256]: cols 0:128 even cand,
                # 128:256 odd cand
                oh = opool.tile([P, TE2 * 2 * DTW], dt.bfloat16, tag="oh")
                ohv = oh[:].rearrange("p (t h s) -> p t h s", h=2, s=DTW)
                iota_v = iota_b[:].rearrange("p (o1 o2 s) -> p o1 o2 s", o1=1, o2=1) \
                                  .broadcast_to((P, TE2, 2, DTW))
                slot_v = stg_s[:, 0:TE2].rearrange("p (t o1 o2) -> p t o1 o2", o1=1, o2=1) \
                                        .broadcast_to((P, TE2, 2, DTW))
                nc.vector.tensor_tensor(out=ohv, in0=iota_v, in1=slot_v,
                                        op=mybir.AluOpType.is_equal)
                wev_v = stg_s[:, TE2:2 * TE2].rearrange("p (t o) -> p t o", o=1) \
                                             .broadcast_to((P, TE2, DTW))
                wod_v = stg_s[:, 2 * TE2:3 * TE2].rearrange("p (t o) -> p t o", o=1) \
                                                 .broadcast_to((P, TE2, DTW))
                nc.vector.tensor_tensor(out=ohv[:, :, 0, :], in0=ohv[:, :, 0, :],
                                        in1=wev_v, op=mybir.AluOpType.mult)
                nc.vector.tensor_tensor(out=ohv[:, :, 1, :], in0=ohv[:, :, 1, :],
                                        in1=wod_v, op=mybir.AluOpType.mult)

                ps = pscat.tile([F, DTW], dt.float32, space="PSUM", tag="ps")
                t = 0
                for m_t, ntile in ms:
                    for tt in range(ntile):
                        mv = m_t[:].rearrange("p (t f) -> p t f", f=PAIRW)
                        for h in range(2):
                            nc.tensor.matmul(
                                out=ps[:],
                                lhsT=mv[:, tt, h * P:h * P + F],
                                rhs=ohv[:, t, h, :],
                                start=(t == 0 and h == 0),
                                stop=(t == TE2 - 1 and h == 1),
                            )
                        t += 1
                nc.vector.tensor_copy(out=dest_fm[0:F, ds(d_expr * DTW, DTW)], in_=ps[:])

            def prop(table_t, dest_fm, tag):
                if "noprop" in cfg:
                    nc.vector.memset(dest_fm[:], 0)
                    return
                with nc.named_scope(f"prop_{tag}"):
                    # pair-row view [NPAIR, 256] of the node-major table
                    table_ap = table_t[:].rearrange("(n two) f -> n (two f)", two=2)
                    # gpsimd pre-touch executes the collective-completion wait
                    pr = spool.tile([1, 2], dt.bfloat16, tag="pr")
                    nc.gpsimd.dma_start(out=pr[:], in_=table_t[0:1, 0:2])
                    with tc.For_i(0, DT - 1, 2) as d0:
                        prop_tile(table_ap, dest_fm, d0)
                        prop_tile(table_ap, dest_fm, d0 + 1)
                    prop_tile(table_ap, dest_fm, DT - 1)

            def dense(layer, tx0_t, t1_t, s2_t, h_t):
                """h = relu(tx0@W0' + t1@W1 + s2@W2') feature-major, bf16."""
                with nc.named_scope(f"dense_{layer}"):
                    wof = layer * 3 * F

                    def chunk(c0, width):
                        pd = pdense.tile([F, NCHUNK], dt.float32, space="PSUM", tag="pd")
                        for ki, rhs_t in enumerate((tx0_t, t1_t, s2_t)):
                            nc.tensor.matmul(
                                out=pd[:, :width],
                                lhsT=w_sb[:, wof + ki * F:wof + (ki + 1) * F],
                                rhs=rhs_t[0:F, ds(c0, width)],
                                start=(ki == 0),
                                stop=(ki == 2),
                            )
                        nc.scalar.activation(
                            out=h_t[0:F, ds(c0, width)], in_=pd[:, :width],
                            func=mybir.ActivationFunctionType.Relu,
                            bias=bias_sb[:, layer:layer + 1],
                        )

                    nfull = SHARD // NCHUNK
                    with tc.For_i(0, nfull * NCHUNK, 2 * NCHUNK) as c0:
                        chunk(c0, NCHUNK)
                        chunk(c0 + NCHUNK, NCHUNK)
                    chunk(nfull * NCHUNK, SHARD - nfull * NCHUNK)

            # ---------- pipeline ----------
            for _r in range(nrep):
                rb = 4 * _r
                fm_to_table(fm["tx0"], rb + 3, "x")       # ag[3] = x table

                prop(ag[rb + 3], fm["t1"], "l1a")         # t1 = L @ x
                fm_to_table(fm["t1"], rb + 0, "t1")
                prop(ag[rb + 0], fm["s2"], "l1b")         # s2 = L @ t1
                dense(0, fm["tx0"], fm["t1"], fm["s2"], fm["h"])
                fm_to_table(fm["h"], rb + 1, "h1")

                prop(ag[rb + 1], fm["t1"], "l2a")
                fm_to_table(fm["t1"], rb + 2, "t1b")
                prop(ag[rb + 2], fm["s2"], "l2b")
                dense(1, fm["h"], fm["t1"], fm["s2"],
                      fm["tx0" if nrep == 1 else "h"])

            hfin = fm["tx0" if nrep == 1 else "h"]
            with nc.named_scope("final"):
                nchunks = (SHARD + NCHUNK - 1) // NCHUNK
                for ci in range(nchunks):
                    c0 = ci * NCHUNK
                    c1 = min(SHARD, c0 + NCHUNK)
                    pf = pdense.tile([2, NCHUNK], dt.float32, space="PSUM", tag="pd")
                    nc.tensor.matmul(out=pf[:, :c1 - c0],
                                     lhsT=w_sb[:, 6 * F:6 * F + 2],
                                     rhs=hfin[0:F, c0:c1], start=True, stop=True)
                    ot = spool.tile([2, NCHUNK], dt.bfloat16, tag="ot")
                    nc.scalar.activation(
                        out=ot[:, :c1 - c0], in_=pf[:, :c1 - c0],
                        func=mybir.ActivationFunctionType.Identity,
                        bias=blin_sb[:],
                    )
                    nc.sync.dma_start(out=out_d[:, c0:c1], in_=ot[:, :c1 - c0])

    nc.compile()
    # memoize the BIR json: bass2jax re-serializes it on every call otherwise
    _json = nc.to_json_bytes()
    nc.to_json_bytes = lambda: _json
    return nc


# --------------------------------------------------------------------------
# input packing
# --------------------------------------------------------------------------
def _pack_inputs(x, edge_index, edge_weight, W1, b1, W2, b2, Wlin, blin):
    new_id, idxp, wp, sp = _preprocess_cached(x, edge_index, edge_weight)

    # int8 x (scale s_x) and u8 |norm_w| (scale s_w, sign folded): the device
    # computes G = sum_e u_e * (.), so L_hat = sigma_w * G with
    # sigma_w = -s_w; scale corrections fold into the dense weights.
    s_x = float(np.abs(x).max()) / 127.0 if np.abs(x).max() > 0 else 1.0
    x_q = np.clip(np.round(x / s_x), -127, 127).astype(np.int8)
    aw = np.abs(wp).max()
    s_w = float(aw) / 255.0 if aw > 0 else 1.0
    sgw = -s_w
    w_u8 = np.clip(np.round(np.abs(wp) / s_w), 0, 255).astype(np.uint8)

    xp = np.zeros((NP_PAD, F), np.int8)
    xp[new_id] = x_q
    xpT = xp.T

    wall = np.concatenate([
        s_x * (W1[0] - W1[2]), s_x * sgw * W1[1], s_x * sgw * sgw * 2.0 * W1[2],
        W2[0] - W2[2], sgw * W2[1], sgw * sgw * 2.0 * W2[2],
    ], axis=1).astype(BF)
    wall = np.concatenate([wall, Wlin.astype(BF)], axis=1)

    # idx wrapped for dma_gather: per dst-tile block of IDXT cols; idx j of the
    # tile's 2176-list sits at (j%16, block + j//16)
    in_maps = []
    for c in range(C):
        idxw = idxp[c].reshape(P, DT, TE2).transpose(1, 2, 0).reshape(DT, IDXT, 16)
        idxw = idxw.transpose(0, 2, 1).reshape(DT * 16, IDXT)  # [(D,16p), IDXT]
        idx_plane = np.zeros((16, IDXCOLS), np.int16)
        for d in range(DT):
            idx_plane[:, d * IDXT:(d + 1) * IDXT] = idxw[d * 16:(d + 1) * 16, :]

        blob = np.zeros((P, CB), BF)
        xs = np.ascontiguousarray(xpT[:, c * SHARD:(c + 1) * SHARD])
        blob[0:F, 0:XC] = xs.view(np.int16).view(BF)
        blob[F:F + 16, 0:XC] = idx_plane[:, 0:XC].view(BF)
        blob[F + 16:F + 32, 0:XC] = idx_plane[:, XC:2 * XC].view(BF)
        blob[0:16, OFF_I2:OFF_I2 + IDX2] = idx_plane[:, 2 * XC:IDXCOLS].view(BF)
        wu = np.zeros((P, SPC * 2), np.uint8)
        wu[:, 0:ECOLS] = w_u8[c]
        blob[:, OFF_W:OFF_W + SPC] = wu.view(np.uint16).view(BF)
        spc = np.zeros((P, SPC * 2), np.uint8)
        spc[:, 0:ECOLS] = sp[c]
        blob[:, OFF_SP:OFF_SP + SPC] = spc.view(np.uint16).view(BF)
        blob[0:F, OFF_WM:OFF_WM + 6 * F + 2] = wall
        blob[0:F, OFF_B:OFF_B + 2] = b1.astype(np.float32).view(np.uint16).view(BF).reshape(F, 2)
        blob[0:F, OFF_B + 2:OFF_B + 4] = b2.astype(np.float32).view(np.uint16).view(BF).reshape(F, 2)
        blob[0:2, OFF_B + 4:OFF_B + 6] = blin.astype(np.float32).view(np.uint16).view(BF).reshape(2, 2)
        in_maps.append({"blob": blob})
    return new_id, in_maps


# --------------------------------------------------------------------------
# entry point
# --------------------------------------------------------------------------
def kernel(x, edge_index, edge_weight, W1, b1, W2, b2, Wlin, blin,
           _trace=False, _tmpdir=None):
    global _compiled
    x = np.asarray(x, np.float32)
    W1 = np.asarray(W1, np.float32); W2 = np.asarray(W2, np.float32)
    b1 = np.asarray(b1, np.float32); b2 = np.asarray(b2, np.float32)
    Wlin = np.asarray(Wlin, np.float32); blin = np.asarray(blin, np.float32)

    new_id, in_maps = _pack_inputs(x, edge_index, edge_weight,
                                   W1, b1, W2, b2, Wlin, blin)

    if _compiled is None:
        _compiled = _build_kernel()
    nc = _compiled

    import time as _time
    _t0 = _time.perf_counter()
    try:
        res = run_bass_kernel_spmd(nc, in_maps, core_ids=list(range(C)),
                                   trace=_trace, tmpdir=_tmpdir)
    except ModuleNotFoundError:
        res = run_bass_kernel_spmd(nc, in_maps, core_ids=list(range(C)),
                                   trace=False, tmpdir=_tmpdir)
    kernel.last_spmd_wall_s = _time.perf_counter() - _t0

    outs_per_core = [np.asarray(res.results[c]["out"]) for c in range(len(res.results))]
    out_p = np.concatenate(outs_per_core, axis=1)   # [2, NP_PAD]
    out = out_p.T[new_id].astype(np.float32)
    if _trace:
        kernel.last_exec_time_ns = res.exec_time_ns
        kernel.last_results = res
    return out
